# revision 4
# baseline (speedup 1.0000x reference)
"""Trainium2 Bass kernel for dense transformer block nn_Block_68221260529679.

Layout: B=2, T=2048, D=2048, N=8 q-heads, K=1 kv-head, H=256, F=16384.

Sharding (8 NeuronCores): DP over batch (2 groups of 4) x TP within group (4).
Core c = 4*b + r: batch b, q-heads {HEADS*r .. HEADS*(r+1)}, MLP hidden rows
[r*FS, (r+1)*FS).  Within each 4-core group:
  - every core computes the full rmsnorm(x) and the shared k/v projection
    (K=1 kv-head) redundantly,
  - attention + o-proj partial summed over the core's heads -> ReduceScatter
    (each core keeps T-slice r),
  - x2 = x + attn on the slice, rmsnorm, transpose -> AllGather of h2^T,
  - gate/up/gelu/down on the F-shard over all T -> ReduceScatter,
  - out slice = x2 + down.  Host assembles the 8 [T/4, D] slices.

All matmuls in bf16 with fp32 PSUM accumulation; norms/softmax/residuals fp32.
The rmsnorm scales (1+scale) and the q scaling H^-0.5 are folded into the
weights host-side; rope sin/cos tables and the additive mask bias are
precomputed host-side per batch.
"""

import numpy as np
import ml_dtypes

import concourse.bass as bass
import concourse.mybir as mybir
import concourse.tile as tile
from concourse import bacc
from concourse.masks import make_identity

F32 = mybir.dt.float32
BF16 = mybir.dt.bfloat16
AF = mybir.ActivationFunctionType
ALU = mybir.AluOpType
BIG_NEG = -2.3819763e38
GROUPS = [[0, 1, 2, 3], [4, 5, 6, 7]]

FULL_CFG = dict(T=2048, D=2048, H=256, HEADS=2, FS=4096)


def build(cfg):
    T, D, H, HEADS, FS = cfg["T"], cfg["D"], cfg["H"], cfg["HEADS"], cfg["FS"]
    assert H == 256
    TP = 4
    TT, DT, FB = T // 128, D // 128, FS // 128
    TCH = T // TP              # attention/MLP T-chunk == rank slice
    NCH, CHT, DCH = T // TCH, TCH // 128, D // TCH
    SLICE = TCH
    ST = SLICE // 128
    QB = HEADS * H // 128      # q col blocks (2 per head)

    nc = bacc.Bacc("TRN2", target_bir_lowering=False, debug=False, num_devices=8)
    x_ext = nc.dram_tensor("x", [T, D], F32, kind="ExternalInput").ap()
    xs_ext = nc.dram_tensor("x_slice", [SLICE, D], F32, kind="ExternalInput").ap()
    wq_ext = nc.dram_tensor("wq", [D, HEADS * H], BF16, kind="ExternalInput").ap()
    wkv_ext = nc.dram_tensor("wkv", [D, 2 * H], BF16, kind="ExternalInput").ap()
    wo_ext = nc.dram_tensor("wo", [HEADS * H, D], BF16, kind="ExternalInput").ap()
    wg_ext = nc.dram_tensor("wg", [D, 2 * FS], BF16, kind="ExternalInput").ap()
    wl_ext = nc.dram_tensor("wl", [FS, D], BF16, kind="ExternalInput").ap()
    sin_ext = nc.dram_tensor("sin", [H // 2, T], F32, kind="ExternalInput").ap()
    cos_ext = nc.dram_tensor("cos", [H // 2, T], F32, kind="ExternalInput").ap()
    mb_ext = nc.dram_tensor("mbias", [T, T], BF16, kind="ExternalInput").ap()
    out_ext = nc.dram_tensor("out", [SLICE, D], F32, kind="ExternalOutput").ap()

    with tile.TileContext(nc) as tc:
        with (
            tc.tile_pool(name="cons", bufs=1) as cons,
            tc.tile_pool(name="acts", bufs=1) as acts,
            tc.tile_pool(name="dram", bufs=1, space="DRAM") as dram,
        ):
            ident = cons.tile([128, 128], BF16)
            make_identity(nc, ident)
            eps = cons.tile([128, 1], F32)
            nc.vector.memset(eps, 1e-6)
            sin_sb = cons.tile([128, T], F32)
            nc.sync.dma_start(out=sin_sb, in_=sin_ext)
            cos_sb = cons.tile([128, T], F32)
            nc.sync.dma_start(out=cos_sb, in_=cos_ext)

            # persistent activations
            qT = acts.tile([128, 2 * HEADS, T], BF16, tag="qT")
            kT = acts.tile([128, 2, T], BF16, tag="kT")
            v_sb = acts.tile([128, TT, H], BF16, tag="v")

            # DRAM intermediates (collectives)
            attn_dram = dram.tile([T, D], F32)
            attn_rs = dram.tile([SLICE, D], F32)
            h2t_own = dram.tile([D, SLICE], BF16)
            h2t_ag = dram.tile([TP * D, SLICE], BF16)
            down_dram = dram.tile([T, D], F32)
            down_rs = dram.tile([SLICE, D], F32)
            x2_dram = dram.tile([SLICE, D], F32)

            # ---------------- P1: rmsnorm(x) + transpose -> hT ----------------
            with (
                tc.tile_pool(name="p12", bufs=2) as p12,
                tc.tile_pool(name="ps12", bufs=2, space="PSUM") as ps12,
            ):
                hT = p12.tile([128, DT, T], BF16, tag="hT", bufs=1)
                for t in range(TT):
                    tsl = slice(t * 128, (t + 1) * 128)
                    xt = p12.tile([128, D], F32, tag="xt")
                    nc.sync.dma_start(out=xt, in_=x_ext[tsl])
                    h = p12.tile([128, D], BF16, tag="h")
                    ssq = p12.tile([128, 1], F32, tag="ssq")
                    nc.scalar.activation(out=h, in_=xt, func=AF.Square,
                                         accum_out=ssq)
                    rstd = p12.tile([128, 1], F32, tag="rstd")
                    nc.scalar.activation(out=rstd, in_=ssq, func=AF.Sqrt,
                                         bias=eps, scale=1.0 / D)
                    nc.vector.reciprocal(out=rstd, in_=rstd)
                    nc.vector.tensor_scalar_mul(h, xt, rstd)
                    for kd in range(DT):
                        pt = ps12.tile([128, 128], BF16, tag="tp")
                        nc.tensor.transpose(pt, h[:, kd * 128:(kd + 1) * 128], ident)
                        nc.vector.tensor_copy(hT[:, kd, tsl], pt)

                # ---------------- P2: q/k/v projections + rope ----------------
                wqs = p12.tile([128, DT, HEADS * H], BF16, tag="wqs", bufs=1)
                nc.sync.dma_start(out=wqs,
                                  in_=wq_ext.rearrange("(a p) n -> p a n", p=128))
                wkvs = p12.tile([128, DT, 2 * H], BF16, tag="wkvs", bufs=1)
                nc.sync.dma_start(out=wkvs,
                                  in_=wkv_ext.rearrange("(a p) n -> p a n", p=128))

                def rope_pair(dst, blk1, blk2, x1p, x2p, csl):
                    cs, sn = cos_sb[:, csl], sin_sb[:, csl]
                    t1 = p12.tile([128, TCH], F32, tag="rp1")
                    t2 = p12.tile([128, TCH], F32, tag="rp2")
                    nc.vector.tensor_tensor(t1, x1p, cs, op=ALU.mult)
                    nc.vector.tensor_tensor(t2, x2p, sn, op=ALU.mult)
                    nc.vector.tensor_tensor(dst[:, blk1, csl], t1, t2,
                                            op=ALU.subtract)
                    nc.vector.tensor_tensor(t1, x2p, cs, op=ALU.mult)
                    nc.vector.tensor_tensor(t2, x1p, sn, op=ALU.mult)
                    nc.vector.tensor_tensor(dst[:, blk2, csl], t1, t2, op=ALU.add)

                for ch in range(NCH):
                    csl = slice(ch * TCH, (ch + 1) * TCH)
                    for hd in range(HEADS):   # q heads
                        p1 = ps12.tile([128, TCH], F32, tag="qk1")
                        p2 = ps12.tile([128, TCH], F32, tag="qk2")
                        for kd in range(DT):
                            c0 = (2 * hd) * 128
                            nc.tensor.matmul(p1, wqs[:, kd, c0:c0 + 128],
                                             hT[:, kd, csl],
                                             start=kd == 0, stop=kd == DT - 1)
                        for kd in range(DT):
                            c1 = (2 * hd + 1) * 128
                            nc.tensor.matmul(p2, wqs[:, kd, c1:c1 + 128],
                                             hT[:, kd, csl],
                                             start=kd == 0, stop=kd == DT - 1)
                        rope_pair(qT, 2 * hd, 2 * hd + 1, p1, p2, csl)
                    # k
                    p1 = ps12.tile([128, TCH], F32, tag="qk1")
                    p2 = ps12.tile([128, TCH], F32, tag="qk2")
                    for kd in range(DT):
                        nc.tensor.matmul(p1, wkvs[:, kd, 0:128], hT[:, kd, csl],
                                         start=kd == 0, stop=kd == DT - 1)
                    for kd in range(DT):
                        nc.tensor.matmul(p2, wkvs[:, kd, 128:256], hT[:, kd, csl],
                                         start=kd == 0, stop=kd == DT - 1)
                    rope_pair(kT, 0, 1, p1, p2, csl)
                    # v (natural layout [S, H])
                    for st in range(ch * CHT, (ch + 1) * CHT):
                        pv = ps12.tile([128, H], F32, tag="vv")
                        for kd in range(DT):
                            nc.tensor.matmul(pv,
                                             hT[:, kd, st * 128:(st + 1) * 128],
                                             wkvs[:, kd, H:2 * H],
                                             start=kd == 0, stop=kd == DT - 1)
                        nc.vector.tensor_copy(v_sb[:, st], pv)

            # ---------------- P3: attention + o-proj ----------------
            with (
                tc.tile_pool(name="p3", bufs=2) as p3,
                tc.tile_pool(name="ps3", bufs=2, space="PSUM") as ps3,
            ):
                wos = p3.tile([128, QB, D], BF16, tag="wos", bufs=1)
                nc.sync.dma_start(out=wos,
                                  in_=wo_ext.rearrange("(a p) d -> p a d", p=128))
                for ch in range(NCH):
                    encT = p3.tile([128, HEADS, 2, TCH], BF16, tag="encT")
                    for hd in range(HEADS):
                        nsc = ch + 1            # causal S-chunks
                        nS = nsc * CHT          # causal S-tiles
                        pT = p3.tile([128, nS, TCH], BF16, tag="pT")
                        for tt in range(CHT):
                            gt = ch * CHT + tt
                            gsl = slice(gt * 128, (gt + 1) * 128)
                            seff = nsc * TCH
                            mb = p3.tile([128, T], BF16, tag="mb")
                            nc.sync.dma_start(out=mb[:, :seff],
                                              in_=mb_ext[gsl, :seff])
                            lg = p3.tile([128, T], F32, tag="lg")
                            for sc in range(nsc):
                                ssl = slice(sc * TCH, (sc + 1) * TCH)
                                pl = ps3.tile([128, TCH], F32, tag="lgp")
                                nc.tensor.matmul(pl, qT[:, 2 * hd, gsl],
                                                 kT[:, 0, ssl],
                                                 start=True, stop=False)
                                nc.tensor.matmul(pl, qT[:, 2 * hd + 1, gsl],
                                                 kT[:, 1, ssl],
                                                 start=False, stop=True)
                                nc.vector.tensor_tensor(lg[:, ssl], pl,
                                                        mb[:, ssl], op=ALU.add)
                            nmax = p3.tile([128, 1], F32, tag="nmax")
                            nc.vector.tensor_reduce(nmax, lg[:, :seff],
                                                    axis=mybir.AxisListType.X,
                                                    op=ALU.max, negate=True)
                            pe = p3.tile([128, T], F32, tag="pexp")
                            sume = p3.tile([128, 1], F32, tag="sume")
                            nc.scalar.activation(out=pe[:, :seff],
                                                 in_=lg[:, :seff], func=AF.Exp,
                                                 bias=nmax, accum_out=sume)
                            rsum = p3.tile([128, 1], F32, tag="rsum")
                            nc.vector.reciprocal(rsum, sume)
                            pb = p3.tile([128, T], BF16, tag="pbf")
                            nc.vector.tensor_scalar_mul(pb[:, :seff],
                                                        pe[:, :seff], rsum)
                            for s in range(nS):
                                ptp = ps3.tile([128, 128], BF16, tag="tp")
                                nc.tensor.transpose(
                                    ptp, pb[:, s * 128:(s + 1) * 128], ident)
                                nc.vector.tensor_copy(
                                    pT[:, s, tt * 128:(tt + 1) * 128], ptp)
                        for m in range(2):
                            pe_ = ps3.tile([128, TCH], F32, tag="enc")
                            for s in range(nS):
                                nc.tensor.matmul(pe_,
                                                 v_sb[:, s, m * 128:(m + 1) * 128],
                                                 pT[:, s, :],
                                                 start=s == 0, stop=s == nS - 1)
                            nc.vector.tensor_copy(encT[:, hd, m], pe_)
                    # o-proj for this chunk, accumulated over the core's heads
                    for tt in range(CHT):
                        gt = ch * CHT + tt
                        ao = p3.tile([128, D], F32, tag="ao")
                        for dch in range(DCH):
                            dsl = slice(dch * TCH, (dch + 1) * TCH)
                            po = ps3.tile([128, TCH], F32, tag="oproj")
                            kk = 0
                            for hd in range(HEADS):
                                for m in range(2):
                                    nc.tensor.matmul(
                                        po,
                                        encT[:, hd, m, tt * 128:(tt + 1) * 128],
                                        wos[:, 2 * hd + m, dsl],
                                        start=kk == 0, stop=kk == 2 * HEADS - 1)
                                    kk += 1
                            nc.vector.tensor_copy(ao[:, dsl], po)
                        nc.sync.dma_start(
                            out=attn_dram[gt * 128:(gt + 1) * 128], in_=ao)

            # ---------------- P4: RS(attn) + residual + norm2 + AG ----------------
            nc.gpsimd.collective_compute(
                "ReduceScatter", ALU.add, replica_groups=GROUPS,
                ins=[attn_dram.opt()], outs=[attn_rs.opt()])
            with (
                tc.tile_pool(name="p4", bufs=2) as p4,
                tc.tile_pool(name="ps4", bufs=2, space="PSUM") as ps4,
            ):
                h2t_sb = p4.tile([128, DT, SLICE], BF16, tag="h2t", bufs=1)
                for t in range(ST):
                    tsl = slice(t * 128, (t + 1) * 128)
                    xt = p4.tile([128, D], F32, tag="xs")
                    nc.sync.dma_start(out=xt, in_=xs_ext[tsl])
                    ar = p4.tile([128, D], F32, tag="ar")
                    nc.sync.dma_start(out=ar, in_=attn_rs[tsl])
                    x2t = p4.tile([128, D], F32, tag="x2t")
                    nc.vector.tensor_tensor(x2t, xt, ar, op=ALU.add)
                    nc.sync.dma_start(out=x2_dram[tsl], in_=x2t)
                    h2 = p4.tile([128, D], BF16, tag="h2")
                    ssq = p4.tile([128, 1], F32, tag="ssq2")
                    nc.scalar.activation(out=h2, in_=x2t, func=AF.Square,
                                         accum_out=ssq)
                    rstd = p4.tile([128, 1], F32, tag="rstd2")
                    nc.scalar.activation(out=rstd, in_=ssq, func=AF.Sqrt,
                                         bias=eps, scale=1.0 / D)
                    nc.vector.reciprocal(out=rstd, in_=rstd)
                    nc.vector.tensor_scalar_mul(h2, x2t, rstd)
                    for kd in range(DT):
                        pt = ps4.tile([128, 128], BF16, tag="tp")
                        nc.tensor.transpose(pt, h2[:, kd * 128:(kd + 1) * 128],
                                            ident)
                        nc.vector.tensor_copy(h2t_sb[:, kd, tsl], pt)
                nc.sync.dma_start(
                    out=h2t_own.rearrange("(a p) s -> p a s", p=128), in_=h2t_sb)
            nc.gpsimd.collective_compute(
                "AllGather", ALU.bypass, replica_groups=GROUPS,
                ins=[h2t_own.opt()], outs=[h2t_ag.opt()])

            # ---------------- P5: MLP on F-shard over all T ----------------
            with (
                tc.tile_pool(name="p5", bufs=2) as p5,
                tc.tile_pool(name="ps5", bufs=2, space="PSUM") as ps5,
            ):
                for r in range(NCH):
                    h2c = p5.tile([128, DT, TCH], BF16, tag="h2c")
                    nc.sync.dma_start(
                        out=h2c,
                        in_=h2t_ag[r * D:(r + 1) * D].rearrange(
                            "(a p) s -> p a s", p=128))
                    ffT = p5.tile([128, FB, TCH], BF16, tag="ffT", bufs=1)
                    for f in range(FB):
                        wgf = p5.tile([128, DT, 256], BF16, tag="wgf", bufs=3)
                        nc.sync.dma_start(
                            out=wgf[:, :, 0:128],
                            in_=wg_ext[:, f * 128:(f + 1) * 128].rearrange(
                                "(a p) n -> p a n", p=128))
                        nc.sync.dma_start(
                            out=wgf[:, :, 128:256],
                            in_=wg_ext[:, FS + f * 128:FS + (f + 1) * 128].rearrange(
                                "(a p) n -> p a n", p=128))
                        gps = ps5.tile([128, TCH], F32, tag="gps")
                        ups = ps5.tile([128, TCH], F32, tag="ups")
                        for kd in range(DT):
                            nc.tensor.matmul(gps, wgf[:, kd, 0:128], h2c[:, kd],
                                             start=kd == 0, stop=kd == DT - 1)
                        for kd in range(DT):
                            nc.tensor.matmul(ups, wgf[:, kd, 128:256], h2c[:, kd],
                                             start=kd == 0, stop=kd == DT - 1)
                        ga = p5.tile([128, TCH], F32, tag="ga")
                        nc.scalar.activation(out=ga, in_=gps,
                                             func=AF.Gelu_apprx_tanh)
                        nc.vector.tensor_tensor(ffT[:, f], ga, ups, op=ALU.mult)
                    for dch in range(DCH):
                        dsl = slice(dch * TCH, (dch + 1) * TCH)
                        wlc = p5.tile([128, FB, TCH], BF16, tag="wlc", bufs=1)
                        nc.sync.dma_start(
                            out=wlc,
                            in_=wl_ext[:, dsl].rearrange("(a p) n -> p a n", p=128))
                        for tt in range(CHT):
                            dps = ps5.tile([128, TCH], F32, tag=f"dps{tt}",
                                           bufs=1)
                            for f in range(FB):
                                nc.tensor.matmul(
                                    dps, ffT[:, f, tt * 128:(tt + 1) * 128],
                                    wlc[:, f],
                                    start=f == 0, stop=f == FB - 1)
                            od = p5.tile([128, TCH], F32, tag="od", bufs=3)
                            nc.vector.tensor_copy(od, dps)
                            gr = (r * CHT + tt) * 128
                            nc.sync.dma_start(
                                out=down_dram[gr:gr + 128, dsl], in_=od)
            nc.gpsimd.collective_compute(
                "ReduceScatter", ALU.add, replica_groups=GROUPS,
                ins=[down_dram.opt()], outs=[down_rs.opt()])

            # ---------------- final residual ----------------
            with tc.tile_pool(name="fin", bufs=2) as fin:
                for t in range(ST):
                    tsl = slice(t * 128, (t + 1) * 128)
                    dr = fin.tile([128, D], F32, tag="dr")
                    nc.sync.dma_start(out=dr, in_=down_rs[tsl])
                    x2t = fin.tile([128, D], F32, tag="x2f")
                    nc.sync.dma_start(out=x2t, in_=x2_dram[tsl])
                    ot = fin.tile([128, D], F32, tag="ot")
                    nc.vector.tensor_tensor(ot, x2t, dr, op=ALU.add)
                    nc.sync.dma_start(out=out_ext[tsl], in_=ot)
    nc.compile()
    return nc


# ---------------------------------------------------------------------------
# host side
# ---------------------------------------------------------------------------

def make_in_maps(cfg, x, positions, attn_mask, scale_attn, w_q, w_kv, w_o,
                 scale_ffn, w_gating, w_linear):
    T, D, H, HEADS, FS = cfg["T"], cfg["D"], cfg["H"], cfg["HEADS"], cfg["FS"]
    SLICE = T // 4
    bf = ml_dtypes.bfloat16
    s1a = (1.0 + np.asarray(scale_attn, np.float32))[:, None]
    s1f = (1.0 + np.asarray(scale_ffn, np.float32))[:, None]
    k_w = (np.asarray(w_kv[0, 0], np.float32) * s1a)
    v_w = (np.asarray(w_kv[1, 0], np.float32) * s1a)
    wkv_h = np.ascontiguousarray(np.concatenate([k_w, v_w], axis=1)).astype(bf)
    freq = 10000.0 ** (2.0 / H * np.arange(H // 2, dtype=np.float32))
    in_maps = []
    for c in range(8):
        b, r = divmod(c, 4)
        hsel = slice(r * HEADS, (r + 1) * HEADS)
        wq_c = np.asarray(w_q[hsel], np.float32) * s1a[None] * H ** -0.5
        wq_c = np.ascontiguousarray(
            np.concatenate(list(wq_c), axis=1)).astype(bf)      # [D, HEADS*H]
        wo_c = np.ascontiguousarray(
            np.concatenate(list(np.asarray(w_o[hsel], np.float32)),
                           axis=0)).astype(bf)                   # [HEADS*H, D]
        fsel = slice(r * FS, (r + 1) * FS)
        wg_c = np.ascontiguousarray(np.concatenate(
            [np.asarray(w_gating[0][:, fsel], np.float32) * s1f,
             np.asarray(w_gating[1][:, fsel], np.float32) * s1f],
            axis=1)).astype(bf)                                  # [D, 2*FS]
        wl_c = np.ascontiguousarray(np.asarray(w_linear[fsel], np.float32)
                                    ).astype(bf)                 # [FS, D]
        pos = np.asarray(positions[b], np.float32)
        rad = pos[None, :] / freq[:, None]                       # [H/2, T]
        mb = np.where(np.asarray(attn_mask[b, 0]), np.float32(0),
                      np.float32(BIG_NEG)).astype(bf)
        xb = np.ascontiguousarray(np.asarray(x[b], np.float32))
        in_maps.append({
            "x": xb,
            "x_slice": np.ascontiguousarray(xb[r * SLICE:(r + 1) * SLICE]),
            "wq": wq_c, "wkv": wkv_h, "wo": wo_c, "wg": wg_c, "wl": wl_c,
            "sin": np.ascontiguousarray(np.sin(rad)),
            "cos": np.ascontiguousarray(np.cos(rad)),
            "mbias": np.ascontiguousarray(mb),
        })
    return in_maps


def assemble(cfg, results, B):
    T, D = cfg["T"], cfg["D"]
    SLICE = T // 4
    out = np.empty((B, T, D), np.float32)
    for c in range(8):
        b, r = divmod(c, 4)
        out[b, r * SLICE:(r + 1) * SLICE] = results[c]["out"]
    return out


# cached compiled program + jitted runner -----------------------------------

_CACHE = {}


def _get_runner(cfg_key, cfg):
    if cfg_key in _CACHE:
        return _CACHE[cfg_key]
    import jax
    from jax.experimental.shard_map import shard_map
    from jax.sharding import Mesh, PartitionSpec
    from concourse import bass2jax

    nc = build(cfg)
    bass2jax.install_neuronx_cc_hook()

    partition_name = (nc.partition_id_tensor.name
                      if nc.partition_id_tensor else None)
    in_names, out_names, out_avals, zero_shapes = [], [], [], []
    for alloc in nc.m.functions[0].allocations:
        if not isinstance(alloc, mybir.MemoryLocationSet):
            continue
        name = alloc.memorylocations[0].name
        if alloc.kind == "ExternalInput":
            if name != partition_name:
                in_names.append(name)
        elif alloc.kind == "ExternalOutput":
            out_names.append(name)
            shape = tuple(alloc.tensor_shape)
            dtype = mybir.dt.np(alloc.dtype)
            out_avals.append(jax.core.ShapedArray(shape, dtype))
            zero_shapes.append((shape, dtype))
    n_params = len(in_names)
    all_in_names = in_names + out_names
    if partition_name is not None:
        all_in_names = all_in_names + [partition_name]

    def _body(*args):
        operands = list(args)
        if partition_name is not None:
            operands.append(bass2jax.partition_id_tensor())
        outs = bass2jax._bass_exec_p.bind(
            *operands,
            out_avals=tuple(out_avals),
            in_names=tuple(all_in_names),
            out_names=tuple(out_names),
            lowering_input_output_aliases=(),
            sim_require_finite=True,
            sim_require_nnan=True,
            nc=nc,
        )
        return tuple(outs)

    n_outs = len(out_names)
    donate = tuple(range(n_params, n_params + n_outs))
    devices = jax.devices()[:8]
    mesh = Mesh(np.asarray(devices), ("core",))
    in_specs = (PartitionSpec("core"),) * (n_params + n_outs)
    out_specs = (PartitionSpec("core"),) * n_outs
    sharded = jax.jit(
        shard_map(_body, mesh=mesh, in_specs=in_specs, out_specs=out_specs,
                  check_rep=False),
        donate_argnums=donate, keep_unused=True)

    def run(in_maps):
        concat_in = [np.concatenate([np.asarray(m[name]) for m in in_maps],
                                    axis=0) for name in in_names]
        zeros = [np.zeros((8 * s[0], *s[1:]), d) for s, d in zero_shapes]
        out_arrs = sharded(*concat_in, *zeros)
        return [
            {name: np.asarray(out_arrs[i]).reshape(8, *out_avals[i].shape)[c]
             for i, name in enumerate(out_names)}
            for c in range(8)
        ]

    _CACHE[cfg_key] = run
    return run


def run_cfg(cfg, inputs):
    cfg_key = tuple(sorted(cfg.items()))
    run = _get_runner(cfg_key, cfg)
    in_maps = make_in_maps(cfg, **inputs)
    results = run(in_maps)
    return assemble(cfg, results, np.asarray(inputs["x"]).shape[0])


def kernel(**inputs):
    return run_cfg(FULL_CFG, inputs)


# revision 5
# speedup vs baseline: 951.2698x; 951.2698x over previous
"""Trainium2 Bass kernel for dense transformer block nn_Block_68221260529679.

Layout: B=2, T=2048, D=2048, N=8 q-heads, K=1 kv-head, H=256, F=16384.

Sharding (8 NeuronCores): DP over batch (2 groups of 4) x TP within group (4).
Core c = 4*b + r: batch b, q-heads {HEADS*r .. HEADS*(r+1)}, MLP hidden rows
[r*FS, (r+1)*FS).  Within each 4-core group:
  - every core computes the full rmsnorm(x) and the shared k/v projection
    (K=1 kv-head) redundantly,
  - attention + o-proj partial summed over the core's heads -> ReduceScatter
    (each core keeps T-slice r),
  - x2 = x + attn on the slice, rmsnorm, transpose -> AllGather of h2^T,
  - gate/up/gelu/down on the F-shard over all T -> ReduceScatter,
  - out slice = x2 + down.  Host assembles the 8 [T/4, D] slices.

All matmuls in bf16 with fp32 PSUM accumulation; norms/softmax/residuals fp32.
The rmsnorm scales (1+scale) and the q scaling H^-0.5 are folded into the
weights host-side; rope sin/cos tables and the additive mask bias are
precomputed host-side per batch.
"""

import numpy as np
import ml_dtypes

import concourse.bass as bass
import concourse.mybir as mybir
import concourse.tile as tile
from concourse import bacc
from concourse.masks import make_identity

F32 = mybir.dt.float32
BF16 = mybir.dt.bfloat16
AF = mybir.ActivationFunctionType
ALU = mybir.AluOpType
BIG_NEG = -2.3819763e38
GROUPS = [[0, 1, 2, 3], [4, 5, 6, 7]]

FULL_CFG = dict(T=2048, D=2048, H=256, HEADS=2, FS=4096)


def build(cfg):
    T, D, H, HEADS, FS = cfg["T"], cfg["D"], cfg["H"], cfg["HEADS"], cfg["FS"]
    assert H == 256
    TP = 4
    TT, DT, FB = T // 128, D // 128, FS // 128
    TCH = T // TP              # attention/MLP T-chunk == rank slice
    NCH, CHT, DCH = T // TCH, TCH // 128, D // TCH
    SLICE = TCH
    ST = SLICE // 128
    QB = HEADS * H // 128      # q col blocks (2 per head)

    nc = bacc.Bacc("TRN2", target_bir_lowering=False, debug=False, num_devices=8)
    x_ext = nc.dram_tensor("x", [T, D], F32, kind="ExternalInput").ap()
    xs_ext = nc.dram_tensor("x_slice", [SLICE, D], F32, kind="ExternalInput").ap()
    wq_ext = nc.dram_tensor("wq", [D, HEADS * H], BF16, kind="ExternalInput").ap()
    wkv_ext = nc.dram_tensor("wkv", [D, 2 * H], BF16, kind="ExternalInput").ap()
    wo_ext = nc.dram_tensor("wo", [HEADS * H, D], BF16, kind="ExternalInput").ap()
    wg_ext = nc.dram_tensor("wg", [D, 2 * FS], BF16, kind="ExternalInput").ap()
    wl_ext = nc.dram_tensor("wl", [FS, D], BF16, kind="ExternalInput").ap()
    sin_ext = nc.dram_tensor("sin", [H // 2, T], F32, kind="ExternalInput").ap()
    cos_ext = nc.dram_tensor("cos", [H // 2, T], F32, kind="ExternalInput").ap()
    mb_ext = nc.dram_tensor("mbias", [T, T], BF16, kind="ExternalInput").ap()
    out_ext = nc.dram_tensor("out", [SLICE, D], F32, kind="ExternalOutput").ap()

    with tile.TileContext(nc) as tc:
        with (
            tc.tile_pool(name="cons", bufs=1) as cons,
            tc.tile_pool(name="acts", bufs=1) as acts,
            tc.tile_pool(name="dram", bufs=1, space="DRAM") as dram,
        ):
            ident = cons.tile([128, 128], BF16)
            make_identity(nc, ident)
            eps = cons.tile([128, 1], F32)
            nc.vector.memset(eps, 1e-6)
            sin_sb = cons.tile([128, T], F32)
            nc.sync.dma_start(out=sin_sb, in_=sin_ext)
            cos_sb = cons.tile([128, T], F32)
            nc.sync.dma_start(out=cos_sb, in_=cos_ext)

            # persistent activations
            qT = acts.tile([128, 2 * HEADS, T], BF16, tag="qT")
            kT = acts.tile([128, 2, T], BF16, tag="kT")
            v_sb = acts.tile([128, TT, H], BF16, tag="v")

            # DRAM intermediates (collectives)
            attn_dram = dram.tile([T, D], F32)
            attn_rs = dram.tile([SLICE, D], F32)
            h2t_own = dram.tile([D, SLICE], BF16)
            h2t_ag = dram.tile([TP * D, SLICE], BF16)
            down_dram = dram.tile([T, D], F32)
            down_rs = dram.tile([SLICE, D], F32)
            x2_dram = dram.tile([SLICE, D], F32)

            # ---------------- P1: rmsnorm(x) + transpose -> hT ----------------
            with (
                tc.tile_pool(name="p12", bufs=2) as p12,
                tc.tile_pool(name="ps12", bufs=2, space="PSUM") as ps12,
            ):
                hT = p12.tile([128, DT, T], BF16, tag="hT", bufs=1)
                for t in range(TT):
                    tsl = slice(t * 128, (t + 1) * 128)
                    xt = p12.tile([128, D], F32, tag="xt")
                    nc.sync.dma_start(out=xt, in_=x_ext[tsl])
                    h = p12.tile([128, D], BF16, tag="h")
                    ssq = p12.tile([128, 1], F32, tag="ssq")
                    nc.scalar.activation(out=h, in_=xt, func=AF.Square,
                                         accum_out=ssq)
                    rstd = p12.tile([128, 1], F32, tag="rstd")
                    nc.scalar.activation(out=rstd, in_=ssq, func=AF.Sqrt,
                                         bias=eps, scale=1.0 / D)
                    nc.vector.reciprocal(out=rstd, in_=rstd)
                    nc.vector.tensor_scalar_mul(h, xt, rstd)
                    for kd in range(DT):
                        pt = ps12.tile([128, 128], BF16, tag="tp")
                        nc.tensor.transpose(pt, h[:, kd * 128:(kd + 1) * 128], ident)
                        nc.vector.tensor_copy(hT[:, kd, tsl], pt)

                # ---------------- P2: q/k/v projections + rope ----------------
                wqs = p12.tile([128, DT, HEADS * H], BF16, tag="wqs", bufs=1)
                nc.sync.dma_start(out=wqs,
                                  in_=wq_ext.rearrange("(a p) n -> p a n", p=128))
                wkvs = p12.tile([128, DT, 2 * H], BF16, tag="wkvs", bufs=1)
                nc.sync.dma_start(out=wkvs,
                                  in_=wkv_ext.rearrange("(a p) n -> p a n", p=128))

                def rope_pair(dst, blk1, blk2, x1p, x2p, csl):
                    cs, sn = cos_sb[:, csl], sin_sb[:, csl]
                    t1 = p12.tile([128, TCH], F32, tag="rp1")
                    t2 = p12.tile([128, TCH], F32, tag="rp2")
                    nc.vector.tensor_tensor(t1, x1p, cs, op=ALU.mult)
                    nc.vector.tensor_tensor(t2, x2p, sn, op=ALU.mult)
                    nc.vector.tensor_tensor(dst[:, blk1, csl], t1, t2,
                                            op=ALU.subtract)
                    nc.vector.tensor_tensor(t1, x2p, cs, op=ALU.mult)
                    nc.vector.tensor_tensor(t2, x1p, sn, op=ALU.mult)
                    nc.vector.tensor_tensor(dst[:, blk2, csl], t1, t2, op=ALU.add)

                for ch in range(NCH):
                    csl = slice(ch * TCH, (ch + 1) * TCH)
                    for hd in range(HEADS):   # q heads
                        p1 = ps12.tile([128, TCH], F32, tag="qk1")
                        p2 = ps12.tile([128, TCH], F32, tag="qk2")
                        for kd in range(DT):
                            c0 = (2 * hd) * 128
                            nc.tensor.matmul(p1, wqs[:, kd, c0:c0 + 128],
                                             hT[:, kd, csl],
                                             start=kd == 0, stop=kd == DT - 1)
                        for kd in range(DT):
                            c1 = (2 * hd + 1) * 128
                            nc.tensor.matmul(p2, wqs[:, kd, c1:c1 + 128],
                                             hT[:, kd, csl],
                                             start=kd == 0, stop=kd == DT - 1)
                        rope_pair(qT, 2 * hd, 2 * hd + 1, p1, p2, csl)
                    # k
                    p1 = ps12.tile([128, TCH], F32, tag="qk1")
                    p2 = ps12.tile([128, TCH], F32, tag="qk2")
                    for kd in range(DT):
                        nc.tensor.matmul(p1, wkvs[:, kd, 0:128], hT[:, kd, csl],
                                         start=kd == 0, stop=kd == DT - 1)
                    for kd in range(DT):
                        nc.tensor.matmul(p2, wkvs[:, kd, 128:256], hT[:, kd, csl],
                                         start=kd == 0, stop=kd == DT - 1)
                    rope_pair(kT, 0, 1, p1, p2, csl)
                    # v (natural layout [S, H])
                    for st in range(ch * CHT, (ch + 1) * CHT):
                        pv = ps12.tile([128, H], F32, tag="vv")
                        for kd in range(DT):
                            nc.tensor.matmul(pv,
                                             hT[:, kd, st * 128:(st + 1) * 128],
                                             wkvs[:, kd, H:2 * H],
                                             start=kd == 0, stop=kd == DT - 1)
                        nc.vector.tensor_copy(v_sb[:, st], pv)

            # ---------------- P3: attention + o-proj ----------------
            with (
                tc.tile_pool(name="p3", bufs=2) as p3,
                tc.tile_pool(name="ps3", bufs=2, space="PSUM") as ps3,
            ):
                wos = p3.tile([128, QB, D], BF16, tag="wos", bufs=1)
                nc.sync.dma_start(out=wos,
                                  in_=wo_ext.rearrange("(a p) d -> p a d", p=128))
                for ch in range(NCH):
                    encT = p3.tile([128, HEADS, 2, TCH], BF16, tag="encT")
                    for hd in range(HEADS):
                        nsc = ch + 1            # causal S-chunks
                        nS = nsc * CHT          # causal S-tiles
                        pT = p3.tile([128, nS, TCH], BF16, tag="pT")
                        for tt in range(CHT):
                            gt = ch * CHT + tt
                            gsl = slice(gt * 128, (gt + 1) * 128)
                            seff = nsc * TCH
                            mb = p3.tile([128, T], BF16, tag="mb")
                            nc.sync.dma_start(out=mb[:, :seff],
                                              in_=mb_ext[gsl, :seff])
                            lg = p3.tile([128, T], F32, tag="lg")
                            for sc in range(nsc):
                                ssl = slice(sc * TCH, (sc + 1) * TCH)
                                pl = ps3.tile([128, TCH], F32, tag="lgp")
                                nc.tensor.matmul(pl, qT[:, 2 * hd, gsl],
                                                 kT[:, 0, ssl],
                                                 start=True, stop=False)
                                nc.tensor.matmul(pl, qT[:, 2 * hd + 1, gsl],
                                                 kT[:, 1, ssl],
                                                 start=False, stop=True)
                                nc.vector.tensor_tensor(lg[:, ssl], pl,
                                                        mb[:, ssl], op=ALU.add)
                            nmax = p3.tile([128, 1], F32, tag="nmax")
                            nc.vector.tensor_reduce(nmax, lg[:, :seff],
                                                    axis=mybir.AxisListType.X,
                                                    op=ALU.max, negate=True)
                            pe = p3.tile([128, T], F32, tag="pexp")
                            sume = p3.tile([128, 1], F32, tag="sume")
                            nc.scalar.activation(out=pe[:, :seff],
                                                 in_=lg[:, :seff], func=AF.Exp,
                                                 bias=nmax, accum_out=sume)
                            rsum = p3.tile([128, 1], F32, tag="rsum")
                            nc.vector.reciprocal(rsum, sume)
                            pb = p3.tile([128, T], BF16, tag="pbf")
                            nc.vector.tensor_scalar_mul(pb[:, :seff],
                                                        pe[:, :seff], rsum)
                            for s in range(nS):
                                ptp = ps3.tile([128, 128], BF16, tag="tp")
                                nc.tensor.transpose(
                                    ptp, pb[:, s * 128:(s + 1) * 128], ident)
                                nc.vector.tensor_copy(
                                    pT[:, s, tt * 128:(tt + 1) * 128], ptp)
                        for m in range(2):
                            pe_ = ps3.tile([128, TCH], F32, tag="enc")
                            for s in range(nS):
                                nc.tensor.matmul(pe_,
                                                 v_sb[:, s, m * 128:(m + 1) * 128],
                                                 pT[:, s, :],
                                                 start=s == 0, stop=s == nS - 1)
                            nc.vector.tensor_copy(encT[:, hd, m], pe_)
                    # o-proj for this chunk, accumulated over the core's heads
                    for tt in range(CHT):
                        gt = ch * CHT + tt
                        ao = p3.tile([128, D], F32, tag="ao")
                        for dch in range(DCH):
                            dsl = slice(dch * TCH, (dch + 1) * TCH)
                            po = ps3.tile([128, TCH], F32, tag="oproj")
                            kk = 0
                            for hd in range(HEADS):
                                for m in range(2):
                                    nc.tensor.matmul(
                                        po,
                                        encT[:, hd, m, tt * 128:(tt + 1) * 128],
                                        wos[:, 2 * hd + m, dsl],
                                        start=kk == 0, stop=kk == 2 * HEADS - 1)
                                    kk += 1
                            nc.vector.tensor_copy(ao[:, dsl], po)
                        nc.sync.dma_start(
                            out=attn_dram[gt * 128:(gt + 1) * 128], in_=ao)

            # ---------------- P4: RS(attn) + residual + norm2 + AG ----------------
            nc.gpsimd.collective_compute(
                "ReduceScatter", ALU.add, replica_groups=GROUPS,
                ins=[attn_dram.opt()], outs=[attn_rs.opt()])
            with (
                tc.tile_pool(name="p4", bufs=2) as p4,
                tc.tile_pool(name="ps4", bufs=2, space="PSUM") as ps4,
            ):
                h2t_sb = p4.tile([128, DT, SLICE], BF16, tag="h2t", bufs=1)
                for t in range(ST):
                    tsl = slice(t * 128, (t + 1) * 128)
                    xt = p4.tile([128, D], F32, tag="xs")
                    nc.sync.dma_start(out=xt, in_=xs_ext[tsl])
                    ar = p4.tile([128, D], F32, tag="ar")
                    nc.sync.dma_start(out=ar, in_=attn_rs[tsl])
                    x2t = p4.tile([128, D], F32, tag="x2t")
                    nc.vector.tensor_tensor(x2t, xt, ar, op=ALU.add)
                    nc.sync.dma_start(out=x2_dram[tsl], in_=x2t)
                    h2 = p4.tile([128, D], BF16, tag="h2")
                    ssq = p4.tile([128, 1], F32, tag="ssq2")
                    nc.scalar.activation(out=h2, in_=x2t, func=AF.Square,
                                         accum_out=ssq)
                    rstd = p4.tile([128, 1], F32, tag="rstd2")
                    nc.scalar.activation(out=rstd, in_=ssq, func=AF.Sqrt,
                                         bias=eps, scale=1.0 / D)
                    nc.vector.reciprocal(out=rstd, in_=rstd)
                    nc.vector.tensor_scalar_mul(h2, x2t, rstd)
                    for kd in range(DT):
                        pt = ps4.tile([128, 128], BF16, tag="tp")
                        nc.tensor.transpose(pt, h2[:, kd * 128:(kd + 1) * 128],
                                            ident)
                        nc.vector.tensor_copy(h2t_sb[:, kd, tsl], pt)
                nc.sync.dma_start(
                    out=h2t_own.rearrange("(a p) s -> p a s", p=128), in_=h2t_sb)
            nc.gpsimd.collective_compute(
                "AllGather", ALU.bypass, replica_groups=GROUPS,
                ins=[h2t_own.opt()], outs=[h2t_ag.opt()])

            # ---------------- P5: MLP on F-shard over all T ----------------
            with (
                tc.tile_pool(name="p5", bufs=2) as p5,
                tc.tile_pool(name="ps5", bufs=2, space="PSUM") as ps5,
            ):
                for r in range(NCH):
                    h2c = p5.tile([128, DT, TCH], BF16, tag="h2c")
                    nc.sync.dma_start(
                        out=h2c,
                        in_=h2t_ag[r * D:(r + 1) * D].rearrange(
                            "(a p) s -> p a s", p=128))
                    ffT = p5.tile([128, FB, TCH], BF16, tag="ffT", bufs=1)
                    for f in range(FB):
                        wgf = p5.tile([128, DT, 256], BF16, tag="wgf", bufs=3)
                        nc.sync.dma_start(
                            out=wgf[:, :, 0:128],
                            in_=wg_ext[:, f * 128:(f + 1) * 128].rearrange(
                                "(a p) n -> p a n", p=128))
                        nc.sync.dma_start(
                            out=wgf[:, :, 128:256],
                            in_=wg_ext[:, FS + f * 128:FS + (f + 1) * 128].rearrange(
                                "(a p) n -> p a n", p=128))
                        gps = ps5.tile([128, TCH], F32, tag="gps")
                        ups = ps5.tile([128, TCH], F32, tag="ups")
                        for kd in range(DT):
                            nc.tensor.matmul(gps, wgf[:, kd, 0:128], h2c[:, kd],
                                             start=kd == 0, stop=kd == DT - 1)
                        for kd in range(DT):
                            nc.tensor.matmul(ups, wgf[:, kd, 128:256], h2c[:, kd],
                                             start=kd == 0, stop=kd == DT - 1)
                        ga = p5.tile([128, TCH], F32, tag="ga")
                        nc.scalar.activation(out=ga, in_=gps,
                                             func=AF.Gelu_apprx_tanh)
                        nc.vector.tensor_tensor(ffT[:, f], ga, ups, op=ALU.mult)
                    for dch in range(DCH):
                        dsl = slice(dch * TCH, (dch + 1) * TCH)
                        wlc = p5.tile([128, FB, TCH], BF16, tag="wlc", bufs=1)
                        nc.sync.dma_start(
                            out=wlc,
                            in_=wl_ext[:, dsl].rearrange("(a p) n -> p a n", p=128))
                        for tt in range(CHT):
                            dps = ps5.tile([128, TCH], F32, tag=f"dps{tt}",
                                           bufs=1)
                            for f in range(FB):
                                nc.tensor.matmul(
                                    dps, ffT[:, f, tt * 128:(tt + 1) * 128],
                                    wlc[:, f],
                                    start=f == 0, stop=f == FB - 1)
                            od = p5.tile([128, TCH], F32, tag="od", bufs=3)
                            nc.vector.tensor_copy(od, dps)
                            gr = (r * CHT + tt) * 128
                            nc.sync.dma_start(
                                out=down_dram[gr:gr + 128, dsl], in_=od)
            nc.gpsimd.collective_compute(
                "ReduceScatter", ALU.add, replica_groups=GROUPS,
                ins=[down_dram.opt()], outs=[down_rs.opt()])

            # ---------------- final residual ----------------
            with tc.tile_pool(name="fin", bufs=2) as fin:
                for t in range(ST):
                    tsl = slice(t * 128, (t + 1) * 128)
                    dr = fin.tile([128, D], F32, tag="dr")
                    nc.sync.dma_start(out=dr, in_=down_rs[tsl])
                    x2t = fin.tile([128, D], F32, tag="x2f")
                    nc.sync.dma_start(out=x2t, in_=x2_dram[tsl])
                    ot = fin.tile([128, D], F32, tag="ot")
                    nc.vector.tensor_tensor(ot, x2t, dr, op=ALU.add)
                    nc.sync.dma_start(out=out_ext[tsl], in_=ot)
    nc.compile()
    return nc


# ---------------------------------------------------------------------------
# host side
# ---------------------------------------------------------------------------

def make_in_maps(cfg, x, positions, attn_mask, scale_attn, w_q, w_kv, w_o,
                 scale_ffn, w_gating, w_linear):
    T, D, H, HEADS, FS = cfg["T"], cfg["D"], cfg["H"], cfg["HEADS"], cfg["FS"]
    SLICE = T // 4
    bf = ml_dtypes.bfloat16
    s1a = (1.0 + np.asarray(scale_attn, np.float32))[:, None]
    s1f = (1.0 + np.asarray(scale_ffn, np.float32))[:, None]
    k_w = (np.asarray(w_kv[0, 0], np.float32) * s1a)
    v_w = (np.asarray(w_kv[1, 0], np.float32) * s1a)
    wkv_h = np.ascontiguousarray(np.concatenate([k_w, v_w], axis=1)).astype(bf)
    freq = 10000.0 ** (2.0 / H * np.arange(H // 2, dtype=np.float32))
    in_maps = []
    for c in range(8):
        b, r = divmod(c, 4)
        hsel = slice(r * HEADS, (r + 1) * HEADS)
        wq_c = np.asarray(w_q[hsel], np.float32) * s1a[None] * H ** -0.5
        wq_c = np.ascontiguousarray(
            np.concatenate(list(wq_c), axis=1)).astype(bf)      # [D, HEADS*H]
        wo_c = np.ascontiguousarray(
            np.concatenate(list(np.asarray(w_o[hsel], np.float32)),
                           axis=0)).astype(bf)                   # [HEADS*H, D]
        fsel = slice(r * FS, (r + 1) * FS)
        wg_c = np.ascontiguousarray(np.concatenate(
            [np.asarray(w_gating[0][:, fsel], np.float32) * s1f,
             np.asarray(w_gating[1][:, fsel], np.float32) * s1f],
            axis=1)).astype(bf)                                  # [D, 2*FS]
        wl_c = np.ascontiguousarray(np.asarray(w_linear[fsel], np.float32)
                                    ).astype(bf)                 # [FS, D]
        pos = np.asarray(positions[b], np.float32)
        rad = pos[None, :] / freq[:, None]                       # [H/2, T]
        mb = np.where(np.asarray(attn_mask[b, 0]), np.float32(0),
                      np.float32(BIG_NEG)).astype(bf)
        xb = np.ascontiguousarray(np.asarray(x[b], np.float32))
        in_maps.append({
            "x": xb,
            "x_slice": np.ascontiguousarray(xb[r * SLICE:(r + 1) * SLICE]),
            "wq": wq_c, "wkv": wkv_h, "wo": wo_c, "wg": wg_c, "wl": wl_c,
            "sin": np.ascontiguousarray(np.sin(rad)),
            "cos": np.ascontiguousarray(np.cos(rad)),
            "mbias": np.ascontiguousarray(mb),
        })
    return in_maps


def assemble(cfg, results, B):
    T, D = cfg["T"], cfg["D"]
    SLICE = T // 4
    out = np.empty((B, T, D), np.float32)
    for c in range(8):
        b, r = divmod(c, 4)
        out[b, r * SLICE:(r + 1) * SLICE] = results[c]["out"]
    return out


# cached compiled program + jitted runner -----------------------------------

_CACHE = {}


def _get_runner(cfg_key, cfg):
    if cfg_key in _CACHE:
        return _CACHE[cfg_key]
    import jax
    from jax.experimental.shard_map import shard_map
    from jax.sharding import Mesh, PartitionSpec
    from concourse import bass2jax

    nc = build(cfg)
    bass2jax.install_neuronx_cc_hook()

    partition_name = (nc.partition_id_tensor.name
                      if nc.partition_id_tensor else None)
    in_names, out_names, out_avals, zero_shapes = [], [], [], []
    for alloc in nc.m.functions[0].allocations:
        if not isinstance(alloc, mybir.MemoryLocationSet):
            continue
        name = alloc.memorylocations[0].name
        if alloc.kind == "ExternalInput":
            if name != partition_name:
                in_names.append(name)
        elif alloc.kind == "ExternalOutput":
            out_names.append(name)
            shape = tuple(alloc.tensor_shape)
            dtype = mybir.dt.np(alloc.dtype)
            out_avals.append(jax.core.ShapedArray(shape, dtype))
            zero_shapes.append((shape, dtype))
    n_params = len(in_names)
    all_in_names = in_names + out_names
    if partition_name is not None:
        all_in_names = all_in_names + [partition_name]

    def _body(*args):
        operands = list(args)
        if partition_name is not None:
            operands.append(bass2jax.partition_id_tensor())
        outs = bass2jax._bass_exec_p.bind(
            *operands,
            out_avals=tuple(out_avals),
            in_names=tuple(all_in_names),
            out_names=tuple(out_names),
            lowering_input_output_aliases=(),
            sim_require_finite=True,
            sim_require_nnan=True,
            nc=nc,
        )
        return tuple(outs)

    n_outs = len(out_names)
    donate = tuple(range(n_params, n_params + n_outs))
    devices = jax.devices()[:8]
    mesh = Mesh(np.asarray(devices), ("core",))
    in_specs = (PartitionSpec("core"),) * (n_params + n_outs)
    out_specs = (PartitionSpec("core"),) * n_outs
    sharded = jax.jit(
        shard_map(_body, mesh=mesh, in_specs=in_specs, out_specs=out_specs,
                  check_rep=False),
        donate_argnums=donate, keep_unused=True)

    class Runner:
        pass

    runner = Runner()
    runner.sharded = sharded
    runner.mesh = mesh
    runner.in_names = in_names
    runner.out_names = out_names
    runner.out_avals = out_avals
    runner.zero_shapes = zero_shapes

    def concat_inputs(in_maps):
        return [np.concatenate([np.asarray(m[name]) for m in in_maps],
                               axis=0) for name in in_names]

    def make_zeros():
        return [np.zeros((8 * s[0], *s[1:]), d) for s, d in zero_shapes]

    def split_outputs(out_arrs):
        return [
            {name: np.asarray(out_arrs[i]).reshape(8, *out_avals[i].shape)[c]
             for i, name in enumerate(out_names)}
            for c in range(8)
        ]

    runner.concat_inputs = concat_inputs
    runner.make_zeros = make_zeros
    runner.split_outputs = split_outputs

    def run(in_maps):
        out_arrs = sharded(*concat_inputs(in_maps), *make_zeros())
        return split_outputs(out_arrs)

    runner.run = run
    _CACHE[cfg_key] = runner
    return runner


def run_cfg(cfg, inputs):
    cfg_key = tuple(sorted(cfg.items()))
    runner = _get_runner(cfg_key, cfg)
    in_maps = make_in_maps(cfg, **inputs)
    results = runner.run(in_maps)
    return assemble(cfg, results, np.asarray(inputs["x"]).shape[0])


def kernel(**inputs):
    return run_cfg(FULL_CFG, inputs)
